# revision 20
# baseline (speedup 1.0000x reference)
"""Trainium2 Bass kernel for nn_CDECF (graph-ODE collaborative filtering).

Contract: kernel(**inputs) takes FULL unsharded numpy inputs (as produced by
reference.setup_inputs()) and returns the FULL [8192] float32 output.

Strategy
--------
The reference scatters the 8192 batch-user embeddings into node rows [0,B)
and batch-item embeddings into rows [NU, NU+B) of a 70000-row node tensor,
runs segment_sum over 2M edges, and reads back only those same rows. Hence
only edges with BOTH endpoints inside those two windows contribute; host
preprocessing compacts the problem to a 16384-row space (~134k edges).

Compact row layout (pairing batch index b with its core): core c owns rows
[2048c, 2048c+2048) = user rows for b in [1024c, 1024c+1024) followed by the
matching item rows. This makes the MLP gate purely core-local.

Per ODE step, per core:
  - dma_gather the messages' source rows from the (replicated, HBM) table
  - scatter-add via one-hot selection-matrix matmuls on the TensorEngine,
    K chunks of 128 edges per 128-row output panel, accumulated in PSUM
  - gate MLP computed from the core-local fp32 slice (overlaps the gather)
  - Euler update in fp32; AllGather republishes the bf16 table
Final scoring (sigmoid of U.I) is a trivial host-side epilogue.
"""
import sys

for _p in ("/opt/trn_rl_repo", "/root/.axon_site/_ro/trn_rl_repo"):
    if _p not in sys.path:
        sys.path.append(_p)

import numpy as np
import ml_dtypes

import concourse.bass as bass
import concourse.bacc as bacc
import concourse.mybir as mybir
import concourse.tile as tile
from concourse import bass_utils
from concourse.masks import make_identity

BF16 = ml_dtypes.bfloat16

NCORES = 8
NU, NI, B, D = 50000, 20000, 8192, 64
ROWS = 2 * B          # 16384 compact rows
SLICE = ROWS // NCORES  # 2048 rows per core
PANEL = 128
NPANEL = SLICE // PANEL  # 16 panels per core
CHUNK = 128           # edges per scatter matmul
GCALL = 1024          # gather idxs per dma_gather call (descriptor-ring cap)
NSTEP = 3

_PROG_CACHE = {}


# ----------------------------------------------------------------------------
# Host preprocessing
# ----------------------------------------------------------------------------

def _compact_rows_user(b):
    return 2048 * (b // 1024) + (b % 1024)


def _compact_rows_item(b):
    return 2048 * (b // 1024) + 1024 + (b % 1024)


def _preprocess_edges(edge_src, edge_dst, edge_vals):
    src = np.asarray(edge_src).astype(np.int64)
    dst = np.asarray(edge_dst).astype(np.int64)
    val = np.asarray(edge_vals).astype(np.float32)

    def in_s(x):
        return (x < B) | ((x >= NU) & (x < NU + B))

    mask = in_s(src) & in_s(dst)
    s, d, v = src[mask], dst[mask], val[mask]

    def compact(ids):
        b_item = ids - NU
        return np.where(ids < B, _compact_rows_user(ids),
                        _compact_rows_item(b_item)).astype(np.int64)

    cs, cd = compact(s), compact(d)

    pg = cs // PANEL                      # global panel id, 0..127
    counts = np.bincount(pg, minlength=ROWS // PANEL)
    K = int(np.ceil(counts.max() / CHUNK))      # chunks per panel
    nchunk = NPANEL * K                          # chunks per core
    nslots = nchunk * CHUNK                      # edge slots per core

    order = np.argsort(pg, kind="stable")
    pg_s = pg[order]
    base = np.zeros(ROWS // PANEL, np.int64)
    base[1:] = np.cumsum(counts)[:-1]
    rank = np.arange(len(order)) - base[pg_s]    # rank within panel
    core_s = (pg_s // NPANEL).astype(np.int64)
    p_s = (pg_s % NPANEL).astype(np.int64)
    # k-major chunk order: chunk c = k*NPANEL + p, so each gather call feeds
    # one accumulation column of EVERY output panel (PSUM-resident scatter).
    k_of = rank // CHUNK
    slot = (k_of * NPANEL + p_s) * CHUNK + rank % CHUNK

    cd_s = cd[order]
    v_s = v[order]
    row_local = (cs[order] % PANEL).astype(np.int64)

    idx_arr = np.zeros((NCORES, nslots), np.int16)
    idx_arr[core_s, slot] = cd_s.astype(np.int16)

    sel = np.zeros((NCORES, nslots, PANEL), np.float32)
    sel[core_s, slot, row_local] = v_s
    # SBUF layout [core, 128 edge-partitions, nchunk*128 row-cols]
    sel = sel.reshape(NCORES, nchunk, CHUNK, PANEL).transpose(0, 2, 1, 3)
    sel = np.ascontiguousarray(sel.reshape(NCORES, CHUNK, nchunk * PANEL))
    sel_bf = sel.astype(BF16)

    # wrapped gather indices: per call block of GCALL slots, wrapped into
    # 16 partitions: wrapped[p, s] = block_idx[s*16 + p]
    assert nslots % GCALL == 0
    ncall = nslots // GCALL
    w = idx_arr.reshape(NCORES, ncall, GCALL // 16, 16).transpose(0, 3, 1, 2)
    # -> [core, 16, ncall, GCALL//16]; concat call blocks along the free axis
    gidx = np.ascontiguousarray(w.reshape(NCORES, 16, ncall * (GCALL // 16)))

    return K, nchunk, nslots, sel_bf, gidx


def _slice_layout(slice_2d):
    """[2048, 64] -> SBUF layout [128, 16*64] (partition = row-in-panel)."""
    return np.ascontiguousarray(
        slice_2d.reshape(NPANEL, PANEL, D).transpose(1, 0, 2).reshape(PANEL,
                                                                      NPANEL * D))


def _unslice_layout(arr):
    """[128, 16*64] -> [2048, 64]."""
    return arr.reshape(PANEL, NPANEL, D).transpose(1, 0, 2).reshape(SLICE, D)


# ----------------------------------------------------------------------------
# Device program
# ----------------------------------------------------------------------------

def _build_program(K, nchunk, nslots, dts):
    FP32 = mybir.dt.float32
    BF = mybir.dt.bfloat16
    nc = bacc.Bacc("TRN2", target_bir_lowering=False, debug=False,
                   num_devices=NCORES, num_swdge_queues=4)

    # --- I/O -----------------------------------------------------------------
    table0 = nc.dram_tensor("table0", [ROWS, 2 * D], BF, kind="ExternalInput")
    slice0 = nc.dram_tensor("slice0", [PANEL, NPANEL * D], FP32,
                            kind="ExternalInput")
    selm_in = nc.dram_tensor("selm", [PANEL, nchunk * PANEL], BF,
                             kind="ExternalInput")
    gidx_in = nc.dram_tensor("gidx", [128, nslots // 16], mybir.dt.int16,
                             kind="ExternalInput")
    w1u_in = nc.dram_tensor("w1u", [D, D], BF, kind="ExternalInput")
    w1i_in = nc.dram_tensor("w1i", [D, D], BF, kind="ExternalInput")
    w2_in = nc.dram_tensor("w2", [D, D], BF, kind="ExternalInput")
    b1_in = nc.dram_tensor("b1", [D, 1], FP32, kind="ExternalInput")
    b2_in = nc.dram_tensor("b2", [D, 1], FP32, kind="ExternalInput")
    outsl = nc.dram_tensor("outslice", [PANEL, NPANEL * D], FP32,
                           kind="ExternalOutput")

    # --- internal DRAM -------------------------------------------------------
    ag_in = [nc.dram_tensor(f"ag_in{s}", [SLICE, 2 * D], BF)
             for s in range(NSTEP - 1)]
    tbl_ag = [nc.dram_tensor(f"tbl_ag{s}", [ROWS, 2 * D], BF,
                             addr_space="Shared") for s in range(NSTEP - 1)]
    warm_in = nc.dram_tensor("warm_in", [16, 16], BF)
    warm_out = nc.dram_tensor("warm_out", [128, 16], BF, addr_space="Shared")

    ncall = nslots // GCALL     # gather calls per step
    cpc = GCALL // CHUNK        # chunks per gather call (8)

    with tile.TileContext(nc) as tc:
        with (
            tc.tile_pool(name="cst", bufs=1) as cst,
            tc.tile_pool(name="state", bufs=1) as state,
            tc.tile_pool(name="work", bufs=2) as work,
            tc.tile_pool(name="psum", bufs=2, space="PSUM") as psum,
            tc.tile_pool(name="psY", bufs=2, space="PSUM") as psYp,
        ):
            # --- persistent tiles -------------------------------------------
            selm = cst.tile([PANEL, nchunk * PANEL], BF)
            gidx = cst.tile([128, nslots // 16], mybir.dt.int16)
            w1u = cst.tile([D, D], BF)
            w1i = cst.tile([D, D], BF)
            w2 = cst.tile([D, D], BF)
            b1 = cst.tile([D, 1], FP32)
            b2 = cst.tile([D, 1], FP32)
            ident = cst.tile([PANEL, PANEL], FP32)
            T = [state.tile([PANEL, NPANEL * D], FP32, name=f"T{i}")
                 for i in range(2)]
            G = [state.tile([PANEL, GCALL], BF, name=f"G{g}")
                 for g in range(ncall)]
            agstage = state.tile([PANEL, NPANEL * 2 * D], BF)
            xTu = state.tile([D, NPANEL // 2 * PANEL], BF)
            xTi = state.tile([D, NPANEL // 2 * PANEL], BF)
            hT = state.tile([D, NPANEL // 2 * PANEL], BF)
            wT = state.tile([D, NPANEL // 2 * PANEL], FP32)
            dtw = state.tile([PANEL, NPANEL // 2 * D], FP32)

            # gidx first: the step-0 gathers wait only on it (table0 is an
            # ExternalInput already resident in HBM); selm isn't needed until
            # the first scatter (~40us in), so it loads behind the gathers.
            nc.sync.dma_start(gidx[:], gidx_in[:])
            nc.sync.dma_start(w1u[:], w1u_in[:])
            nc.sync.dma_start(w1i[:], w1i_in[:])
            nc.sync.dma_start(w2[:], w2_in[:])
            nc.sync.dma_start(b1[:], b1_in[:])
            nc.sync.dma_start(b2[:], b2_in[:])
            nc.sync.dma_start(T[0][:], slice0[:])
            nc.sync.dma_start(selm[:], selm_in[:])
            nc.vector.memset(agstage[:], 0.0)
            make_identity(nc, ident[:])
            # warm up the ncfw collective path under the step-0 gathers so the
            # first real AllGather doesn't pay the cold trigger delay.
            nc.gpsimd.collective_compute(
                "AllGather",
                mybir.AluOpType.bypass,
                replica_groups=[list(range(NCORES))],
                ins=[warm_in.ap().opt()],
                outs=[warm_out.ap().opt()],
            )

            NB = NPANEL // 2 * PANEL    # local batch (1024)

            for step in range(NSTEP):
                dt = float(dts[step])
                Tcur = T[step % 2]
                Tnxt = T[(step + 1) % 2]
                tbl = table0 if step == 0 else tbl_ag[step - 1]

                # ---- gather edge source rows (1024-idx calls over the 4
                # SWDGE queues; each queue pair generates concurrently) ------
                for g in range(ncall):
                    nc.gpsimd.dma_gather(
                        out_ap=G[g][:].rearrange("p (c e) -> p c e", e=2 * D),
                        in_ap=tbl.ap(),
                        idxs_ap=gidx[:, g * (GCALL // 16):(g + 1) * (GCALL // 16)],
                        num_idxs=GCALL,
                        num_idxs_reg=GCALL,
                        elem_size=2 * D,
                        queue_num=g % 4,
                    )

                # ---- gate MLP from local fp32 slice (core-local) -----------
                for p in range(NPANEL):
                    tp = psum.tile([D, PANEL], FP32, tag="tps")
                    nc.tensor.transpose(tp[:], Tcur[:, p * D:(p + 1) * D],
                                        ident[:])
                    dst = xTu if p < NPANEL // 2 else xTi
                    q = p % (NPANEL // 2)
                    nc.scalar.copy(dst[:, q * PANEL:(q + 1) * PANEL], tp[:])
                for chix in range(2):
                    sl = slice(chix * 512, (chix + 1) * 512)
                    hp = psum.tile([D, 512], FP32, tag="mlp")
                    nc.tensor.matmul(hp[:], w1u[:], xTu[:, sl],
                                     start=True, stop=False)
                    nc.tensor.matmul(hp[:], w1i[:], xTi[:, sl],
                                     start=False, stop=True)
                    nc.scalar.activation(hT[:, sl], hp[:],
                                         mybir.ActivationFunctionType.Relu,
                                         bias=b1[:])
                    zp = psum.tile([D, 512], FP32, tag="mlp")
                    nc.tensor.matmul(zp[:], w2[:], hT[:, sl],
                                     start=True, stop=True)
                    nc.scalar.activation(wT[:, sl], zp[:],
                                         mybir.ActivationFunctionType.Sigmoid,
                                         bias=b2[:])
                for q in range(NPANEL // 2):
                    gp = psum.tile([PANEL, D], FP32, tag="tps")
                    nc.tensor.transpose(gp[:], wT[:, q * PANEL:(q + 1) * PANEL],
                                        ident[:D, :D])
                    nc.scalar.mul(dtw[:, q * D:(q + 1) * D], gp[:], dt)

                # ---- scatter (one-hot matmuls, k-major): every gather call
                # feeds one accumulation column of all 16 panels; the whole
                # step accumulates into one persistent PSUM region ----------
                psY = psYp.tile([PANEL, NPANEL * D], FP32, tag="psY")
                for k in range(K):
                    for p in range(NPANEL):
                        c = k * NPANEL + p       # chunk id within core
                        g, ci = divmod(c, cpc)
                        nc.tensor.matmul(
                            psY[:, p * D:(p + 1) * D],
                            selm[:, c * PANEL:(c + 1) * PANEL],
                            G[g][:, ci * 2 * D:ci * 2 * D + D],
                            start=(k == 0), stop=(k == K - 1),
                        )

                # ---- batched Euler update (DVE reads PSUM directly) --------
                HB = NPANEL // 2 * D    # 512
                for h in range(2):
                    sl = slice(h * HB, (h + 1) * HB)
                    eff = work.tile([PANEL, HB], FP32, tag="eff")
                    nc.vector.tensor_tensor(eff[:], psY[:, sl], Tcur[:, sl],
                                            op=mybir.AluOpType.subtract)
                    nc.vector.tensor_tensor(eff[:], eff[:], dtw[:],
                                            op=mybir.AluOpType.mult)
                    nc.vector.tensor_tensor(Tnxt[:, sl], Tcur[:, sl], eff[:],
                                            op=mybir.AluOpType.add)
                if step < NSTEP - 1:
                    nc.vector.tensor_copy(
                        agstage[:].rearrange("j (p f) -> j p f",
                                             f=2 * D)[:, :, 0:D],
                        Tnxt[:].rearrange("j (p f) -> j p f", f=D))

                # ---- publish updated table / final output ------------------
                if step < NSTEP - 1:
                    nc.sync.dma_start(
                        ag_in[step].ap().rearrange("(p j) f -> j p f", j=PANEL),
                        agstage[:].rearrange("j (p f) -> j p f", f=2 * D))
                    nc.gpsimd.collective_compute(
                        "AllGather",
                        mybir.AluOpType.bypass,
                        replica_groups=[list(range(NCORES))],
                        ins=[ag_in[step].ap().opt()],
                        outs=[tbl_ag[step].ap().opt()],
                    )
                else:
                    nc.sync.dma_start(outsl.ap(), Tnxt[:])

    nc.compile()
    return nc


# ----------------------------------------------------------------------------
# Entry point
# ----------------------------------------------------------------------------

def kernel(users, items, user_emb, item_emb, w1, b1, w2, b2,
           edge_src, edge_dst, edge_vals, time_steps):
    users = np.asarray(users)
    items = np.asarray(items)
    user_emb = np.asarray(user_emb, dtype=np.float32)
    item_emb = np.asarray(item_emb, dtype=np.float32)
    w1 = np.asarray(w1, dtype=np.float32)
    b1 = np.asarray(b1, dtype=np.float32)
    w2 = np.asarray(w2, dtype=np.float32)
    b2 = np.asarray(b2, dtype=np.float32)
    time_steps = np.asarray(time_steps, dtype=np.float32)
    dts = np.diff(time_steps)

    # initial compact table
    E_u = user_emb[users]
    E_i = item_emb[items]
    bidx = np.arange(B)
    rows_u = _compact_rows_user(bidx)
    rows_i = _compact_rows_item(bidx)
    table0 = np.zeros((ROWS, D), np.float32)
    table0[rows_u] = E_u
    table0[rows_i] = E_i
    table0_pad = np.zeros((ROWS, 2 * D), BF16)
    table0_pad[:, :D] = table0.astype(BF16)

    K, nchunk, nslots, sel_bf, gidx = _preprocess_edges(
        edge_src, edge_dst, edge_vals)

    key = (K, nchunk, nslots, tuple(np.round(dts, 9).tolist()))
    if key not in _PROG_CACHE:
        _PROG_CACHE[key] = _build_program(K, nchunk, nslots, dts)
    nc = _PROG_CACHE[key]

    w1u = np.ascontiguousarray(w1[:D]).astype(BF16)
    w1i = np.ascontiguousarray(w1[D:]).astype(BF16)
    w2b = w2.astype(BF16)
    b1c = np.ascontiguousarray(b1.reshape(D, 1))
    b2c = np.ascontiguousarray(b2.reshape(D, 1))

    in_maps = []
    for c in range(NCORES):
        sl = table0[c * SLICE:(c + 1) * SLICE]
        in_maps.append({
            "table0": table0_pad,
            "slice0": _slice_layout(sl),
            "selm": sel_bf[c],
            "gidx": np.tile(gidx[c], (8, 1)),
            "w1u": w1u, "w1i": w1i, "w2": w2b, "b1": b1c, "b2": b2c,
        })

    res = bass_utils.run_bass_kernel_spmd(
        nc, in_maps, core_ids=list(range(NCORES)),
        trace=False)
    kernel.last_results = res

    final = np.zeros((ROWS, D), np.float32)
    for c in range(NCORES):
        final[c * SLICE:(c + 1) * SLICE] = _unslice_layout(
            res.results[c]["outslice"])

    Uf = final[rows_u]
    If = final[rows_i]
    logits = np.sum(Uf * If, axis=1)
    return (1.0 / (1.0 + np.exp(-logits))).astype(np.float32)



# revision 30
# speedup vs baseline: 1.2716x; 1.2716x over previous
"""Trainium2 Bass kernel for nn_CDECF (graph-ODE collaborative filtering).

Contract: kernel(**inputs) takes FULL unsharded numpy inputs (as produced by
reference.setup_inputs()) and returns the FULL [8192] float32 output.

Strategy
--------
The reference scatters the 8192 batch-user embeddings into node rows [0,B)
and batch-item embeddings into rows [NU, NU+B) of a 70000-row node tensor,
runs segment_sum over 2M edges, and reads back only those same rows. Hence
only edges with BOTH endpoints inside those two windows contribute; host
preprocessing compacts the problem to a 16384-row space (~134k edges).

Compact row layout (pairing batch index b with its core): core c owns rows
[2048c, 2048c+2048) = user rows for b in [1024c, 1024c+1024) followed by the
matching item rows. This makes the MLP gate purely core-local.

Per ODE step, per core:
  - dma_gather the messages' source rows from the (replicated, HBM) table
  - scatter-add via one-hot selection-matrix matmuls on the TensorEngine,
    K chunks of 128 edges per 128-row output panel, accumulated in PSUM
  - gate MLP computed from the core-local fp32 slice (overlaps the gather)
  - Euler update in fp32; AllGather republishes the bf16 table
Final scoring (sigmoid of U.I) is a trivial host-side epilogue.
"""
import sys

for _p in ("/opt/trn_rl_repo", "/root/.axon_site/_ro/trn_rl_repo"):
    if _p not in sys.path:
        sys.path.append(_p)

import numpy as np
import ml_dtypes

import concourse.bass as bass
import concourse.bacc as bacc
import concourse.mybir as mybir
import concourse.tile as tile
from concourse import bass_utils
from concourse.masks import make_identity

BF16 = ml_dtypes.bfloat16

NCORES = 8
NU, NI, B, D = 50000, 20000, 8192, 64
ROWS = 2 * B          # 16384 compact rows
SLICE = ROWS // NCORES  # 2048 rows per core
PANEL = 128
NPANEL = SLICE // PANEL  # 16 panels per core
CHUNK = 128           # edges per scatter matmul
GCALL = 1024          # gather idxs per dma_gather call (descriptor-ring cap)
NSTEP = 3

_PROG_CACHE = {}


# ----------------------------------------------------------------------------
# Host preprocessing
# ----------------------------------------------------------------------------

def _compact_rows_user(b):
    return 2048 * (b // 1024) + (b % 1024)


def _compact_rows_item(b):
    return 2048 * (b // 1024) + 1024 + (b % 1024)


def _preprocess_edges(edge_src, edge_dst, edge_vals):
    src = np.asarray(edge_src).astype(np.int64)
    dst = np.asarray(edge_dst).astype(np.int64)
    val = np.asarray(edge_vals).astype(np.float32)

    def in_s(x):
        return (x < B) | ((x >= NU) & (x < NU + B))

    mask = in_s(src) & in_s(dst)
    s, d, v = src[mask], dst[mask], val[mask]

    def compact(ids):
        b_item = ids - NU
        return np.where(ids < B, _compact_rows_user(ids),
                        _compact_rows_item(b_item)).astype(np.int64)

    cs, cd = compact(s), compact(d)

    pg = cs // PANEL                      # global panel id, 0..127
    counts = np.bincount(pg, minlength=ROWS // PANEL)
    K = int(np.ceil(counts.max() / CHUNK))      # chunks per panel
    nchunk = NPANEL * K                          # chunks per core
    nslots = nchunk * CHUNK                      # edge slots per core

    order = np.argsort(pg, kind="stable")
    pg_s = pg[order]
    base = np.zeros(ROWS // PANEL, np.int64)
    base[1:] = np.cumsum(counts)[:-1]
    rank = np.arange(len(order)) - base[pg_s]    # rank within panel
    core_s = (pg_s // NPANEL).astype(np.int64)
    p_s = (pg_s % NPANEL).astype(np.int64)
    chunk_s = p_s * K + rank // CHUNK            # panel-major chunk id

    cd_s = cd[order]
    v_s = v[order]
    row_local = (cs[order] % PANEL).astype(np.int64)

    # gather-table POSITION order: pos(core, p, j) = core*2048 + j*16 + p.
    # Adjacent positions pair adjacent panels at the same partition, so the
    # (unpadded bf16) table is fetched as 256B PAIRS; the scatter picks the
    # edge's row via parity-split sel matrices (parity == dst panel parity).
    pos_all = (cd_s // SLICE) * SLICE + (cd_s % PANEL) * NPANEL \
        + (cd_s % SLICE) // PANEL

    # sort slots within each (core, chunk) by gathered pair for HBM locality
    # on the SDMA drain (the sel matrix absorbs any within-chunk permutation)
    sec = np.lexsort((pos_all, chunk_s, core_s))
    core2 = core_s[sec]
    chunk2 = chunk_s[sec]
    key = core2 * nchunk + chunk2
    cstart = np.zeros(NCORES * nchunk + 1, np.int64)
    np.add.at(cstart, key + 1, 1)
    cstart = np.cumsum(cstart)
    rank2 = np.arange(len(key)) - cstart[key]
    slot = chunk2 * CHUNK + rank2

    idx_arr = np.zeros((NCORES, nslots), np.int16)
    idx_arr[core2, slot] = (pos_all[sec] >> 1).astype(np.int16)

    sel = np.zeros((NCORES, nslots, 2, PANEL), np.float32)
    sel[core2, slot, pos_all[sec] & 1, row_local[sec]] = v_s[sec]
    # SBUF layout [core, 128 edge-partitions, nchunk*(even|odd)*128 row-cols]
    sel = sel.reshape(NCORES, nchunk, CHUNK, 2 * PANEL).transpose(0, 2, 1, 3)
    sel = np.ascontiguousarray(sel.reshape(NCORES, CHUNK, nchunk * 2 * PANEL))
    sel_bf = sel.astype(BF16)

    # wrapped gather indices: per call block of GCALL slots, wrapped into
    # 16 partitions: wrapped[p, s] = block_idx[s*16 + p]
    assert nslots % GCALL == 0
    ncall = nslots // GCALL
    w = idx_arr.reshape(NCORES, ncall, GCALL // 16, 16).transpose(0, 3, 1, 2)
    # -> [core, 16, ncall, GCALL//16]; concat call blocks along the free axis
    gidx = np.ascontiguousarray(w.reshape(NCORES, 16, ncall * (GCALL // 16)))

    return K, nchunk, nslots, sel_bf, gidx


def _slice_layout(slice_2d):
    """[2048, 64] -> SBUF layout [128, 16*64] (partition = row-in-panel)."""
    return np.ascontiguousarray(
        slice_2d.reshape(NPANEL, PANEL, D).transpose(1, 0, 2).reshape(PANEL,
                                                                      NPANEL * D))


def _unslice_layout(arr):
    """[128, 16*64] -> [2048, 64]."""
    return arr.reshape(PANEL, NPANEL, D).transpose(1, 0, 2).reshape(SLICE, D)


# ----------------------------------------------------------------------------
# Device program
# ----------------------------------------------------------------------------

def _build_program(K, nchunk, nslots, dts):
    FP32 = mybir.dt.float32
    BF = mybir.dt.bfloat16
    nc = bacc.Bacc("TRN2", target_bir_lowering=False, debug=False,
                   num_devices=NCORES, num_swdge_queues=4)

    # --- I/O -----------------------------------------------------------------
    table0 = nc.dram_tensor("table0", [ROWS, 2 * D], BF, kind="ExternalInput")
    slice0 = nc.dram_tensor("slice0", [PANEL, NPANEL * D], FP32,
                            kind="ExternalInput")
    selm_in = nc.dram_tensor("selm", [PANEL, nchunk * PANEL], BF,
                             kind="ExternalInput")
    gidx_in = nc.dram_tensor("gidx", [128, nslots // 16], mybir.dt.int16,
                             kind="ExternalInput")
    w1u_in = nc.dram_tensor("w1u", [D, D], BF, kind="ExternalInput")
    w1i_in = nc.dram_tensor("w1i", [D, D], BF, kind="ExternalInput")
    w2_in = nc.dram_tensor("w2", [D, D], BF, kind="ExternalInput")
    b1_in = nc.dram_tensor("b1", [D, 1], FP32, kind="ExternalInput")
    b2_in = nc.dram_tensor("b2", [D, 1], FP32, kind="ExternalInput")
    outsl = nc.dram_tensor("outslice", [PANEL, NPANEL * D], FP32,
                           kind="ExternalOutput")

    # --- internal DRAM -------------------------------------------------------
    ag_in = [nc.dram_tensor(f"ag_in{s}", [SLICE, 2 * D], BF)
             for s in range(NSTEP - 1)]
    tbl_ag = [nc.dram_tensor(f"tbl_ag{s}", [ROWS, 2 * D], BF,
                             addr_space="Shared") for s in range(NSTEP - 1)]
    warm_in = nc.dram_tensor("warm_in", [16, 16], BF)
    warm_out = nc.dram_tensor("warm_out", [128, 16], BF, addr_space="Shared")

    ncall = nslots // GCALL     # gather calls per step
    cpc = GCALL // CHUNK        # chunks per gather call (8)

    with tile.TileContext(nc) as tc:
        with (
            tc.tile_pool(name="cst", bufs=1) as cst,
            tc.tile_pool(name="state", bufs=1) as state,
            tc.tile_pool(name="work", bufs=2) as work,
            tc.tile_pool(name="psum", bufs=2, space="PSUM") as psum,
        ):
            # --- persistent tiles -------------------------------------------
            selm = cst.tile([PANEL, nchunk * PANEL], BF)
            gidx = cst.tile([128, nslots // 16], mybir.dt.int16)
            w1u = cst.tile([D, D], BF)
            w1i = cst.tile([D, D], BF)
            w2 = cst.tile([D, D], BF)
            b1 = cst.tile([D, 1], FP32)
            b2 = cst.tile([D, 1], FP32)
            ident = cst.tile([PANEL, PANEL], FP32)
            T = [state.tile([PANEL, NPANEL * D], FP32, name=f"T{i}")
                 for i in range(2)]
            G = [state.tile([PANEL, GCALL], BF, name=f"G{g}")
                 for g in range(ncall)]
            agstage = state.tile([PANEL, NPANEL * 2 * D], BF)
            xTu = state.tile([D, NPANEL // 2 * PANEL], BF)
            xTi = state.tile([D, NPANEL // 2 * PANEL], BF)
            hT = state.tile([D, NPANEL // 2 * PANEL], BF)
            wT = state.tile([D, NPANEL // 2 * PANEL], FP32)
            dtw = state.tile([PANEL, NPANEL // 2 * D], FP32)
            Ysb = state.tile([PANEL, NPANEL * D], FP32)

            # gidx first: the step-0 gathers wait only on it (table0 is an
            # ExternalInput already resident in HBM); selm isn't needed until
            # the first scatter (~40us in), so it loads behind the gathers.
            nc.sync.dma_start(gidx[:], gidx_in[:])
            nc.sync.dma_start(w1u[:], w1u_in[:])
            nc.sync.dma_start(w1i[:], w1i_in[:])
            nc.sync.dma_start(w2[:], w2_in[:])
            nc.sync.dma_start(b1[:], b1_in[:])
            nc.sync.dma_start(b2[:], b2_in[:])
            nc.sync.dma_start(T[0][:], slice0[:])
            nc.sync.dma_start(selm[:], selm_in[:])
            nc.vector.memset(agstage[:], 0.0)
            make_identity(nc, ident[:])
            # warm up the ncfw collective path under the step-0 gathers so the
            # first real AllGather doesn't pay the cold trigger delay.
            nc.gpsimd.collective_compute(
                "AllGather",
                mybir.AluOpType.bypass,
                replica_groups=[list(range(NCORES))],
                ins=[warm_in.ap().opt()],
                outs=[warm_out.ap().opt()],
            )

            NB = NPANEL // 2 * PANEL    # local batch (1024)

            for step in range(NSTEP):
                dt = float(dts[step])
                Tcur = T[step % 2]
                Tnxt = T[(step + 1) % 2]
                tbl = table0 if step == 0 else tbl_ag[step - 1]

                # ---- gather edge source rows (1024-idx calls over the 4
                # SWDGE queues; each queue pair generates concurrently) ------
                for g in range(ncall):
                    nc.gpsimd.dma_gather(
                        out_ap=G[g][:].rearrange("p (c e) -> p c e", e=2 * D),
                        in_ap=tbl.ap(),
                        idxs_ap=gidx[:, g * (GCALL // 16):(g + 1) * (GCALL // 16)],
                        num_idxs=GCALL,
                        num_idxs_reg=GCALL,
                        elem_size=2 * D,
                        queue_num=g % 4,
                    )

                # ---- gate MLP from local fp32 slice (core-local) -----------
                for p in range(NPANEL):
                    tp = psum.tile([D, PANEL], FP32, tag="tps")
                    nc.tensor.transpose(tp[:], Tcur[:, p * D:(p + 1) * D],
                                        ident[:])
                    dst = xTu if p < NPANEL // 2 else xTi
                    q = p % (NPANEL // 2)
                    nc.scalar.copy(dst[:, q * PANEL:(q + 1) * PANEL], tp[:])
                for chix in range(2):
                    sl = slice(chix * 512, (chix + 1) * 512)
                    hp = psum.tile([D, 512], FP32, tag="mlp")
                    nc.tensor.matmul(hp[:], w1u[:], xTu[:, sl],
                                     start=True, stop=False)
                    nc.tensor.matmul(hp[:], w1i[:], xTi[:, sl],
                                     start=False, stop=True)
                    nc.scalar.activation(hT[:, sl], hp[:],
                                         mybir.ActivationFunctionType.Relu,
                                         bias=b1[:])
                    zp = psum.tile([D, 512], FP32, tag="mlp")
                    nc.tensor.matmul(zp[:], w2[:], hT[:, sl],
                                     start=True, stop=True)
                    nc.scalar.activation(wT[:, sl], zp[:],
                                         mybir.ActivationFunctionType.Sigmoid,
                                         bias=b2[:])
                for q in range(NPANEL // 2):
                    gp = psum.tile([PANEL, D], FP32, tag="tps")
                    nc.tensor.transpose(gp[:], wT[:, q * PANEL:(q + 1) * PANEL],
                                        ident[:D, :D])
                    nc.scalar.mul(dtw[:, q * D:(q + 1) * D], gp[:], dt)

                # ---- scatter (one-hot matmuls); ACT drains PSUM -> Y -------
                for p in range(NPANEL):
                    ps = psum.tile([PANEL, D], FP32, tag="ps")
                    for k in range(K):
                        c = p * K + k            # chunk id within core
                        g, ci = divmod(c, cpc)
                        nc.tensor.matmul(
                            ps[:],
                            selm[:, c * PANEL:(c + 1) * PANEL],
                            G[g][:, ci * 2 * D:ci * 2 * D + D],
                            start=(k == 0), stop=(k == K - 1),
                        )
                    nc.scalar.copy(Ysb[:, p * D:(p + 1) * D], ps[:])

                # ---- batched Euler update (SBUF-only DVE, two halves) ------
                HB = NPANEL // 2 * D    # 512
                for h in range(2):
                    sl = slice(h * HB, (h + 1) * HB)
                    eff = work.tile([PANEL, HB], FP32, tag="eff")
                    nc.vector.tensor_tensor(eff[:], Ysb[:, sl], Tcur[:, sl],
                                            op=mybir.AluOpType.subtract)
                    nc.vector.tensor_tensor(eff[:], eff[:], dtw[:],
                                            op=mybir.AluOpType.mult)
                    nc.vector.tensor_tensor(Tnxt[:, sl], Tcur[:, sl], eff[:],
                                            op=mybir.AluOpType.add)
                if step < NSTEP - 1:
                    nc.vector.tensor_copy(
                        agstage[:].rearrange("j (p f) -> j p f",
                                             f=2 * D)[:, :, 0:D],
                        Tnxt[:].rearrange("j (p f) -> j p f", f=D))

                # ---- publish updated table / final output ------------------
                if step < NSTEP - 1:
                    nc.sync.dma_start(
                        ag_in[step].ap().rearrange("(p j) f -> j p f", j=PANEL),
                        agstage[:].rearrange("j (p f) -> j p f", f=2 * D))
                    nc.gpsimd.collective_compute(
                        "AllGather",
                        mybir.AluOpType.bypass,
                        replica_groups=[list(range(NCORES))],
                        ins=[ag_in[step].ap().opt()],
                        outs=[tbl_ag[step].ap().opt()],
                    )
                else:
                    nc.sync.dma_start(outsl.ap(), Tnxt[:])

    nc.compile()
    return nc


# ----------------------------------------------------------------------------
# Entry point
# ----------------------------------------------------------------------------

def kernel(users, items, user_emb, item_emb, w1, b1, w2, b2,
           edge_src, edge_dst, edge_vals, time_steps):
    users = np.asarray(users)
    items = np.asarray(items)
    user_emb = np.asarray(user_emb, dtype=np.float32)
    item_emb = np.asarray(item_emb, dtype=np.float32)
    w1 = np.asarray(w1, dtype=np.float32)
    b1 = np.asarray(b1, dtype=np.float32)
    w2 = np.asarray(w2, dtype=np.float32)
    b2 = np.asarray(b2, dtype=np.float32)
    time_steps = np.asarray(time_steps, dtype=np.float32)
    dts = np.diff(time_steps)

    # initial compact table
    E_u = user_emb[users]
    E_i = item_emb[items]
    bidx = np.arange(B)
    rows_u = _compact_rows_user(bidx)
    rows_i = _compact_rows_item(bidx)
    table0 = np.zeros((ROWS, D), np.float32)
    table0[rows_u] = E_u
    table0[rows_i] = E_i
    table0_pad = np.zeros((ROWS, 2 * D), BF16)
    table0_pad[:, :D] = table0.astype(BF16)

    K, nchunk, nslots, sel_bf, gidx = _preprocess_edges(
        edge_src, edge_dst, edge_vals)

    key = (K, nchunk, nslots, tuple(np.round(dts, 9).tolist()))
    if key not in _PROG_CACHE:
        _PROG_CACHE[key] = _build_program(K, nchunk, nslots, dts)
    nc = _PROG_CACHE[key]

    w1u = np.ascontiguousarray(w1[:D]).astype(BF16)
    w1i = np.ascontiguousarray(w1[D:]).astype(BF16)
    w2b = w2.astype(BF16)
    b1c = np.ascontiguousarray(b1.reshape(D, 1))
    b2c = np.ascontiguousarray(b2.reshape(D, 1))

    in_maps = []
    for c in range(NCORES):
        sl = table0[c * SLICE:(c + 1) * SLICE]
        in_maps.append({
            "table0": table0_pad,
            "slice0": _slice_layout(sl),
            "selm": sel_bf[c],
            "gidx": np.tile(gidx[c], (8, 1)),
            "w1u": w1u, "w1i": w1i, "w2": w2b, "b1": b1c, "b2": b2c,
        })

    res = bass_utils.run_bass_kernel_spmd(
        nc, in_maps, core_ids=list(range(NCORES)),
        trace=False)
    kernel.last_results = res

    final = np.zeros((ROWS, D), np.float32)
    for c in range(NCORES):
        final[c * SLICE:(c + 1) * SLICE] = _unslice_layout(
            res.results[c]["outslice"])

    Uf = final[rows_u]
    If = final[rows_i]
    logits = np.sum(Uf * If, axis=1)
    return (1.0 / (1.0 + np.exp(-logits))).astype(np.float32)

